# revision 2
# baseline (speedup 1.0000x reference)
"""Trainium2 Bass kernel for nn_CustomModel_21019569946955 (pendulum Lyapunov loss).

Data-parallel over 8 NeuronCores: each core processes B/8 = 8192 samples with
replicated MLP weights. Fused single-pass pipeline:

  fwd (bf16):  h1 = tanh(W1^T x^T); u1 = 1-h1^2
               h2 = tanh(W2^T h1); [y_pred; V] = W3^T h2
  bwd (fp8 DoubleRow + constant split):
               g1 = u1 * (r - W2 (w3 . h2^2)),  r = W2 w3  (exact, prologue)
               dVdx = W1 g1
  The constant split shrinks the fp8 quantization error ~3x: the moving
  operand w3.h2^2 has much less energy than w3.(1-h2^2).
  final stage: batch-major pendulum ODE + penalties + partial sums for the
  scalar custom_loss (combined on host: pure data-parallel mean).
"""
import numpy as np
import concourse.bass as bass
import concourse.tile as tile
from concourse import bacc, mybir
from concourse.bass_utils import run_bass_kernel_spmd
from concourse.masks import make_identity

F32 = mybir.dt.float32
BF16 = mybir.dt.bfloat16
F8E4 = mybir.dt.float8e4
AF = mybir.ActivationFunctionType
ALU = mybir.AluOpType
DR = mybir.MatmulPerfMode.DoubleRow

# problem constants (hardcoded from the reference)
G = 9.8
L, I_, MB, MC, AT, AR = 0.3, 2.0, 1.0, 3.0, 0.2, 0.2
C1 = L * MB            # 0.3
C2 = I_ + L * L * MB   # 2.09
C3 = MB + MC           # 4.0
PEN = 10000.0
ALPHA = 0.1
EPS = 1e-7
C1SQ = C1 * C1
C2C3 = C2 * C3

B, H, D = 65536, 2048, 4
NCORES = 8
BC = B // NCORES        # 8192 samples per core
N = 512                 # batch-chunk (moving free dim)
CH = BC // N            # 16 chunks
KT = H // 128           # 16 feature tiles
FB = BC // 128          # 64 samples per partition in the final stage

SW = 64.0               # fp8 scale for W2^T
SG = 64.0               # fp8 scale for the moving operand w3.h2^2
INV = 1.0 / (SW * SG)

# fp32 round-to-nearest-int trick + Cody-Waite 2pi for sin/cos range reduction
RC = float(1.5 * 2 ** 23)
INV2PI = float(1.0 / (2.0 * np.pi))
TWOPI_HI = float(np.float32(2.0 * np.pi))
TWOPI_LO = float(2.0 * np.pi - np.float64(np.float32(2.0 * np.pi)))
HALFPI = float(np.pi / 2)

_NC_CACHE = {}


def build():
    nc = bacc.Bacc("TRN2", target_bir_lowering=False, debug=False)

    xd = nc.declare_dram_parameter("x", [BC, D], F32, isOutput=False)
    yd = nc.declare_dram_parameter("y", [BC], F32, isOutput=False)
    W1d = nc.declare_dram_parameter("W1", [D, H], F32, isOutput=False)
    b1d = nc.declare_dram_parameter("b1", [H], F32, isOutput=False)
    W2d = nc.declare_dram_parameter("W2", [H, H], F32, isOutput=False)
    b2d = nc.declare_dram_parameter("b2", [H], F32, isOutput=False)
    W3d = nc.declare_dram_parameter("W3", [H, 2], F32, isOutput=False)
    b3d = nc.declare_dram_parameter("b3", [2], F32, isOutput=False)

    loss_out = nc.declare_dram_parameter("loss_pen", [BC], F32, isOutput=True)
    part_out = nc.declare_dram_parameter("partials", [128, 2], F32, isOutput=True)

    with tile.TileContext(nc) as tc:
        with tc.tile_pool(name="dram", bufs=1, space="DRAM") as dpool:
            yv_d = dpool.tile([2, BC], F32, tag="yv_d", name="yv_d")
            dv_d = dpool.tile([D, BC], F32, tag="dv_d", name="dv_d")

            with tc.tile_pool(name="wpool", bufs=1) as wpool, \
                 tc.tile_pool(name="small", bufs=1) as small:

                # ---- persistent big tiles ----
                w2f = wpool.tile([128, KT, H], BF16, tag="w2f", name="w2f")
                w2b8 = wpool.tile([128, KT, H], F8E4, tag="w2b8", name="w2b8")
                h1 = wpool.tile([128, KT, N], BF16, tag="h1", name="h1")
                u1 = wpool.tile([128, KT, N], BF16, tag="u1", name="u1")
                g2m8 = wpool.tile([128, KT, N], F8E4, tag="g2m8", name="g2m8")

                # ---- small weights / constants ----
                w1sb = small.tile([D, H], BF16, tag="w1sb", name="w1sb")
                w1t = small.tile([128, KT, D], BF16, tag="w1t", name="w1t")
                w3sb = small.tile([128, KT, 2], BF16, tag="w3sb", name="w3sb")
                identb = small.tile([128, 128], BF16, tag="identb", name="identb")

                with tc.tile_pool(name="cvt", bufs=1) as cvt:
                    w1f = cvt.tile([D, H], F32, tag="w1f", name="w1f")
                    nc.sync.dma_start(w1f[:], W1d[:, :])
                    nc.vector.tensor_copy(w1sb[:], w1f[:])

                    w1tf = cvt.tile([128, KT, D], F32, tag="w1tf", name="w1tf")
                    for k in range(KT):
                        nc.gpsimd.dma_start(
                            w1tf[:, k],
                            W1d[:, k * 128:(k + 1) * 128].rearrange("d p -> p d"))
                    nc.vector.tensor_copy(w1t[:], w1tf[:])

                    w3f = cvt.tile([128, KT, 2], F32, tag="w3f", name="w3f")
                    nc.gpsimd.dma_start(w3f[:], W3d.rearrange("(k p) j -> p k j", p=128))
                    nc.vector.tensor_copy(w3sb[:], w3f[:])

                    ident_f = cvt.tile([128, 128], F32, tag="ident_f", name="ident_f")
                    make_identity(nc, ident_f[:])
                    nc.vector.tensor_copy(identb[:], ident_f[:])

                b1c = small.tile([128, KT], F32, tag="b1c", name="b1c")
                nc.gpsimd.dma_start(b1c[:], b1d.rearrange("(k p) -> p k", p=128))
                b2c = small.tile([128, KT], F32, tag="b2c", name="b2c")
                nc.gpsimd.dma_start(b2c[:], b2d.rearrange("(k p) -> p k", p=128))
                b3c = small.tile([2, 1], F32, tag="b3c", name="b3c")
                nc.gpsimd.dma_start(b3c[:], b3d.rearrange("(p o) -> p o", o=1))
                w3c1 = small.tile([128, KT, 1], F32, tag="w3c1", name="w3c1")
                nc.gpsimd.dma_start(
                    w3c1[:], W3d.rearrange("(k p) j -> p k j", p=128)[:, :, 1:2])
                # -SG * w3[:,1] per-partition scalars for the fp8 moving operand
                negw3s = small.tile([128, KT], F32, tag="negw3s", name="negw3s")
                nc.vector.tensor_scalar_mul(negw3s[:], w3c1[:, :, 0], -SG)
                # rS = SG*SW * (W2 @ w3col), accumulated during the W2 load
                rS = small.tile([128, KT], F32, tag="rS", name="rS")

                with tc.tile_pool(name="tmp", bufs=2) as tmp, \
                     tc.tile_pool(name="pp", bufs=2, space="PSUM") as pp:

                    # ---- W2 load + convert + transpose (prologue) ----
                    HQ = H // 4
                    with tc.tile_pool(name="w2cv", bufs=2) as cvp:
                        # w3 broadcast across partitions for the r matvec
                        w3row = cvp.tile([1, H], F32, tag="w3row", name="w3row",
                                         bufs=1)
                        nc.sync.dma_start(
                            w3row[:], W3d[:, 1:2].rearrange("h o -> o h"))
                        w3bc = cvp.tile([128, H], BF16, tag="w3bc", name="w3bc",
                                        bufs=1)
                        w3bcf = cvp.tile([128, H], F32, tag="w3bcf", name="w3bcf",
                                         bufs=1)
                        nc.gpsimd.partition_broadcast(w3bcf[:], w3row[:])
                        nc.vector.tensor_copy(w3bc[:], w3bcf[:])
                        r4 = cvp.tile([128, KT, 4], F32, tag="r4", name="r4",
                                      bufs=1)

                        # column-quarter-major: chunk 0's first fwd groups only
                        # need the low column blocks, so they start ~1/4 in
                        for hh in range(4):
                            hq = slice(hh * HQ, (hh + 1) * HQ)
                            for k in range(KT):
                                t = cvp.tile([128, HQ], F32, tag="w2tmp",
                                             name="w2tmp")
                                nc.sync.dma_start(
                                    t[:], W2d[k * 128:(k + 1) * 128, hq])
                                nc.vector.tensor_copy(w2f[:, k, hq], t[:])
                                # r partial: sum_j W2[128k+p, j] * w3[j]
                                prod = cvp.tile([128, HQ], F32, tag="prod",
                                                name="prod")
                                nc.gpsimd.tensor_mul(prod[:], w2f[:, k, hq],
                                                     w3bc[:, hq])
                                junk = cvp.tile([128, HQ], BF16, tag="junk",
                                                name="junk")
                                nc.scalar.activation(
                                    junk[:], prod[:], AF.Copy, bias=0.0,
                                    accum_out=r4[:, k, hh:hh + 1])
                                # W2^T tiles (fp8, scaled): for this quarter
                                for m in range(4 * hh, 4 * hh + 4):
                                    trp = pp.tile([128, 128], BF16, tag="gps",
                                                  name="trp", bufs=2)
                                    nc.tensor.transpose(
                                        trp[:], w2f[:, k, m * 128:(m + 1) * 128],
                                        identb[:])
                                    nc.vector.tensor_scalar_mul(
                                        w2b8[:, m, k * 128:(k + 1) * 128],
                                        trp[:], SW)

                        # rS = SG*SW * sum_hh r4
                        ra = cvp.tile([128, KT], F32, tag="ra", name="ra", bufs=1)
                        rb = cvp.tile([128, KT], F32, tag="rb", name="rb", bufs=1)
                        nc.vector.tensor_add(ra[:], r4[:, :, 0], r4[:, :, 1])
                        nc.vector.tensor_add(rb[:], r4[:, :, 2], r4[:, :, 3])
                        nc.vector.tensor_add(ra[:], ra[:], rb[:])
                        nc.vector.tensor_scalar_mul(rS[:], ra[:], SG * SW)

                    # ---- prologue: x chunk 0 + h1/u1 chunk 0 ----
                    def load_x(i):
                        xtf = tmp.tile([D, N], F32, tag="xtf", name="xtf", bufs=1)
                        nc.gpsimd.dma_start(
                            xtf[:], xd[i * N:(i + 1) * N, :].rearrange("n d -> d n"))
                        xt = tmp.tile([D, N], BF16, tag="xt", name="xt")
                        nc.vector.tensor_copy(xt[:], xtf[:])
                        return xt

                    def fwd1(m1, xt):
                        # h1/u1 for feature block m1 from xt (chunk's transposed x)
                        hps = pp.tile([128, N], F32, tag="h1ps", name="hps", bufs=2)
                        nc.tensor.matmul(hps[:], w1sb[:, m1 * 128:(m1 + 1) * 128],
                                         xt[:], start=True, stop=True)
                        nc.scalar.activation(h1[:, m1], hps[:], AF.Tanh,
                                             bias=b1c[:, m1:m1 + 1])
                        nc.gpsimd.tensor_mul(u1[:, m1], h1[:, m1], h1[:, m1])
                        nc.gpsimd.tensor_scalar(u1[:, m1], u1[:, m1], -1.0, 1.0,
                                                ALU.mult, ALU.add)

                    xt_cur = load_x(0)
                    for m1 in range(KT):
                        fwd1(m1, xt_cur)

                    # ---- main loop over chunks ----
                    for i in range(CH):
                        # ---- phase A: fwd W2 / W3 / g2m ----
                        yvp = pp.tile([2, N], F32, tag="yvdv", name="yvp", bufs=1)
                        h2t_prev = None
                        for m2 in range(KT):
                            ps = pp.tile([128, N], F32, tag="ps", name="ps", bufs=3)
                            for k in range(KT):
                                nc.tensor.matmul(
                                    ps[:], w2f[:, k, m2 * 128:(m2 + 1) * 128],
                                    h1[:, k], start=(k == 0), stop=(k == KT - 1))
                            if m2 > 0:
                                nc.tensor.matmul(yvp[:], w3sb[:, m2 - 1],
                                                 h2t_prev[:],
                                                 start=(m2 == 1), stop=False)
                            h2t = tmp.tile([128, N], BF16, tag="h2t", name="h2t",
                                           bufs=2)
                            nc.scalar.activation(h2t[:], ps[:], AF.Tanh,
                                                 bias=b2c[:, m2:m2 + 1])
                            sq = tmp.tile([128, N], BF16, tag="sq", name="sq",
                                          bufs=2)
                            nc.gpsimd.tensor_mul(sq[:], h2t[:], h2t[:])
                            # moving operand: -SG * w3 * h2^2 quantized to fp8
                            nc.vector.tensor_scalar(
                                g2m8[:, m2], sq[:], negw3s[:, m2:m2 + 1], None,
                                ALU.mult)
                            h2t_prev = h2t
                        nc.tensor.matmul(yvp[:], w3sb[:, KT - 1], h2t_prev[:],
                                         start=False, stop=True)
                        yvt = tmp.tile([2, N], F32, tag="yvt", name="yvt", bufs=1)
                        nc.vector.tensor_scalar(yvt[:], yvp[:], b3c[:], None,
                                                ALU.add)
                        nc.sync.dma_start(yv_d[:, i * N:(i + 1) * N], yvt[:])

                        # x for next chunk
                        if i + 1 < CH:
                            xt_cur = load_x(i + 1)

                        # ---- phase C: bwd fp8 DoubleRow / dVdx (+ fwd W1 of
                        # chunk i+1) ----
                        dvp = pp.tile([D, N], F32, tag="yvdv", name="dvp", bufs=1)
                        g1h_prev = None
                        NP = KT // 2
                        for m1 in range(KT):
                            gps = pp.tile([128, N], F32, tag="gps", name="gps", bufs=2)
                            torder = (list(range(NP)) if m1 < KT - 1
                                      else list(range(NP - 1, -1, -1)))
                            for j, t2 in enumerate(torder):
                                nc.tensor.matmul(
                                    gps[:],
                                    w2b8[:, 2 * t2:2 * t2 + 2,
                                         m1 * 128:(m1 + 1) * 128],
                                    g2m8[:, 2 * t2:2 * t2 + 2, :],
                                    start=(j == 0), stop=(j == NP - 1),
                                    perf_mode=DR)
                            if m1 > 0:
                                nc.tensor.matmul(dvp[:], w1t[:, m1 - 1],
                                                 g1h_prev[:],
                                                 start=(m1 == 1), stop=False)
                            # g1 = u1 * (r + psum/(SG*SW))
                            gt = tmp.tile([128, N], F32, tag="gt", name="gt",
                                          bufs=2)
                            nc.vector.tensor_scalar(gt[:], gps[:],
                                                    rS[:, m1:m1 + 1], INV,
                                                    ALU.add, ALU.mult)
                            g1h = tmp.tile([128, N], BF16, tag="g1h", name="g1h",
                                           bufs=2)
                            nc.vector.tensor_mul(g1h[:], gt[:], u1[:, m1])
                            # interleaved fwd W1 for chunk i+1 (overwrites h1/u1[m1]
                            # after their last chunk-i use)
                            if i + 1 < CH:
                                fwd1(m1, xt_cur)
                            g1h_prev = g1h
                        nc.tensor.matmul(dvp[:], w1t[:, KT - 1], g1h_prev[:],
                                         start=False, stop=True)
                        dvt = tmp.tile([D, N], F32, tag="dvt", name="dvt", bufs=1)
                        nc.vector.tensor_copy(dvt[:], dvp[:])
                        nc.sync.dma_start(dv_d[:, i * N:(i + 1) * N], dvt[:])

                # ---- final stage: batch-major per-sample math ----
                with tc.tile_pool(name="fpool", bufs=1) as fpool:
                    _final_stage(nc, tc, fpool, xd, yd, yv_d, dv_d,
                                 loss_out, part_out)

    nc.compile()
    return nc


def _final_stage(nc, tc, fpool, xd, yd, yv_d, dv_d, loss_out, part_out):
    def plane_from_row(dram_row_ap, tag):
        t = fpool.tile([128, FB], F32, tag=tag, name=tag)
        nc.sync.dma_start(t[:], dram_row_ap.rearrange("(p f) -> p f", p=128))
        return t

    ypred = plane_from_row(yv_d[0], "ypred")
    vpl = plane_from_row(yv_d[1], "vpl")
    dv0 = plane_from_row(dv_d[0], "dv0")
    dv1 = plane_from_row(dv_d[1], "dv1")
    dv2 = plane_from_row(dv_d[2], "dv2")
    dv3 = plane_from_row(dv_d[3], "dv3")
    ypl = plane_from_row(yd[:], "ypl")

    xpl = fpool.tile([128, FB, D], F32, tag="xpl", name="xpl")
    nc.sync.dma_start(xpl[:], xd.rearrange("(p f) d -> p f d", p=128))
    x2 = xpl[:, :, 1]
    x3 = xpl[:, :, 2]
    x4 = xpl[:, :, 3]

    zc = fpool.tile([128, 1], F32, tag="zc", name="zc")
    nc.vector.memset(zc[:], 0.0)

    def ftile(tag):
        return fpool.tile([128, FB], F32, tag=tag, name=tag)

    def sin_reduced(src_ap, negate, bias, tag):
        # sin(bias + (negate ? -src : src)), range-reduced mod 2pi
        w = ftile(tag + "w")
        nc.vector.tensor_scalar(w[:], src_ap, -1.0 if negate else 1.0, bias,
                                ALU.mult, ALU.add)
        t = ftile(tag + "t")
        nc.vector.tensor_scalar(t[:], w[:], INV2PI, RC, ALU.mult, ALU.add)
        r = ftile(tag + "r")
        nc.vector.tensor_scalar(r[:], t[:], RC, None, ALU.subtract)
        a = ftile(tag + "a")
        nc.vector.scalar_tensor_tensor(a[:], r[:], -TWOPI_HI, w[:], ALU.mult, ALU.add)
        y_ = ftile(tag + "y")
        nc.vector.scalar_tensor_tensor(y_[:], r[:], -TWOPI_LO, a[:], ALU.mult, ALU.add)
        o = ftile(tag + "o")
        nc.scalar.activation(o[:], y_[:], AF.Sin, bias=zc[:])
        return o

    s = sin_reduced(x3, False, 0.0, "s")
    c = sin_reduced(x3, True, HALFPI, "c")     # cos(x) = sin(pi/2 - x)

    f = ftile("f")
    nc.vector.scalar_tensor_tensor(f[:], x2, -AT, ypred[:], ALU.mult, ALU.add)

    u = ftile("u")
    nc.vector.tensor_mul(u[:], c[:], c[:])
    den = ftile("den")
    nc.vector.tensor_scalar(den[:], u[:], -C1SQ, C2C3, ALU.mult, ALU.add)
    rden = ftile("rden")
    nc.vector.reciprocal(rden[:], den[:])

    cs = ftile("cs")
    nc.vector.tensor_mul(cs[:], c[:], s[:])
    x4sq = ftile("x4sq")
    nc.vector.tensor_mul(x4sq[:], x4, x4)
    cx4 = ftile("cx4")
    nc.vector.tensor_mul(cx4[:], c[:], x4)
    sx4sq = ftile("sx4sq")
    nc.vector.tensor_mul(sx4sq[:], s[:], x4sq[:])
    csx4sq = ftile("csx4sq")
    nc.vector.tensor_mul(csx4sq[:], cs[:], x4sq[:])
    cf = ftile("cf")
    nc.vector.tensor_mul(cf[:], c[:], f[:])

    # x2p = (G*C1^2*c*s + C2*f - AR*C1*c*x4 - C1*C2*s*x4^2) / den
    p1 = ftile("p1")
    nc.vector.tensor_scalar(p1[:], f[:], C2, None, ALU.mult)
    nc.vector.scalar_tensor_tensor(p1[:], cs[:], G * C1SQ, p1[:], ALU.mult, ALU.add)
    nc.vector.scalar_tensor_tensor(p1[:], cx4[:], -AR * C1, p1[:], ALU.mult, ALU.add)
    nc.vector.scalar_tensor_tensor(p1[:], sx4sq[:], -C1 * C2, p1[:], ALU.mult, ALU.add)
    x2p = ftile("x2p")
    nc.vector.tensor_mul(x2p[:], p1[:], rden[:])

    # x4p = (G*C1*C3*s + C1*c*f - AR*C3*x4 - C1^2*c*s*x4^2) / den
    p2 = ftile("p2")
    nc.vector.tensor_scalar(p2[:], s[:], G * C1 * C3, None, ALU.mult)
    nc.vector.scalar_tensor_tensor(p2[:], cf[:], C1, p2[:], ALU.mult, ALU.add)
    nc.vector.scalar_tensor_tensor(p2[:], x4, -AR * C3, p2[:], ALU.mult, ALU.add)
    nc.vector.scalar_tensor_tensor(p2[:], csx4sq[:], -C1SQ, p2[:], ALU.mult, ALU.add)
    x4p = ftile("x4p")
    nc.vector.tensor_mul(x4p[:], p2[:], rden[:])

    # Vdot = dV . [x2, x2p, x4, x4p]
    vd = ftile("vd")
    nc.vector.tensor_mul(vd[:], dv0[:], x2)
    t_ = ftile("vt")
    nc.vector.tensor_mul(t_[:], dv1[:], x2p[:])
    nc.vector.tensor_add(vd[:], vd[:], t_[:])
    nc.vector.tensor_mul(t_[:], dv2[:], x4)
    nc.vector.tensor_add(vd[:], vd[:], t_[:])
    nc.vector.tensor_mul(t_[:], dv3[:], x4p[:])
    nc.vector.tensor_add(vd[:], vd[:], t_[:])

    # penalties: PEN*relu(-V) + PEN*relu(Vdot)
    pen = ftile("pen")
    nc.vector.tensor_scalar(pen[:], vpl[:], 0.0, -PEN, ALU.min, ALU.mult)
    pen2 = ftile("pen2")
    nc.vector.tensor_scalar(pen2[:], vd[:], 0.0, PEN, ALU.max, ALU.mult)
    nc.vector.tensor_add(pen[:], pen[:], pen2[:])
    nc.sync.dma_start(loss_out.rearrange("(p f) -> p f", p=128), pen[:])

    # partial sums for custom_loss: sum(d^2), sum((y - y_pred)^2)
    ypc = ftile("ypc")
    nc.vector.tensor_scalar(ypc[:], ypred[:], EPS, None, ALU.max)
    l1 = ftile("l1")
    nc.scalar.activation(l1[:], ypc[:], AF.Ln, bias=1.0)
    yc = ftile("yc")
    nc.vector.tensor_scalar(yc[:], ypl[:], EPS, None, ALU.max)
    l2 = ftile("l2")
    nc.scalar.activation(l2[:], yc[:], AF.Ln, bias=1.0)
    dd = ftile("dd")
    nc.vector.tensor_sub(dd[:], l1[:], l2[:])
    d2s = fpool.tile([128, 1], F32, tag="d2s", name="d2s")
    dtmp = ftile("dtmp")
    nc.scalar.activation(dtmp[:], dd[:], AF.Square, bias=zc[:], accum_out=d2s[:])
    ee = ftile("ee")
    nc.vector.tensor_sub(ee[:], ypl[:], ypred[:])
    es = fpool.tile([128, 1], F32, tag="es", name="es")
    nc.scalar.activation(dtmp[:], ee[:], AF.Square, bias=zc[:], accum_out=es[:])

    parts = fpool.tile([128, 2], F32, tag="parts", name="parts")
    nc.vector.tensor_copy(parts[:, 0:1], d2s[:])
    nc.vector.tensor_copy(parts[:, 1:2], es[:])
    nc.sync.dma_start(part_out[:, :], parts[:])


def kernel(**inputs):
    x = np.ascontiguousarray(inputs["x"], dtype=np.float32)
    y = np.ascontiguousarray(inputs["y"], dtype=np.float32)
    W1 = np.ascontiguousarray(inputs["W1"], dtype=np.float32)
    b1 = np.ascontiguousarray(inputs["b1"], dtype=np.float32)
    W2 = np.ascontiguousarray(inputs["W2"], dtype=np.float32)
    b2 = np.ascontiguousarray(inputs["b2"], dtype=np.float32)
    W3 = np.ascontiguousarray(inputs["W3"], dtype=np.float32)
    b3 = np.ascontiguousarray(inputs["b3"], dtype=np.float32)

    if "nc" not in _NC_CACHE:
        _NC_CACHE["nc"] = build()
    nc = _NC_CACHE["nc"]

    in_maps = []
    for cid in range(NCORES):
        sl = slice(cid * BC, (cid + 1) * BC)
        in_maps.append({
            "x": x[sl], "y": y[sl],
            "W1": W1, "b1": b1, "W2": W2, "b2": b2, "W3": W3, "b3": b3,
        })
    res = run_bass_kernel_spmd(nc, in_maps, list(range(NCORES)))

    loss = np.concatenate([res.results[c]["loss_pen"] for c in range(NCORES)])
    parts = np.stack([res.results[c]["partials"] for c in range(NCORES)])
    sums = parts.astype(np.float64).sum(axis=(0, 1))
    scalar = ALPHA * sums[0] / B + (1.0 - ALPHA) * sums[1] / B
    return (loss + np.float32(scalar)).astype(np.float32)


# revision 6
# speedup vs baseline: 1.1751x; 1.1751x over previous
"""Trainium2 Bass kernel for nn_CustomModel_21019569946955 (pendulum Lyapunov loss).

Data-parallel over 8 NeuronCores: each core processes B/8 = 8192 samples with
replicated MLP weights. Fused single-pass pipeline:

  fwd (bf16):  h1 = tanh(W1^T x^T); u1 = 1-h1^2
               h2 = tanh(W2^T h1); [y_pred; V] = W3^T h2
  bwd (fp8 DoubleRow + constant split):
               g1 = u1 * (r - W2 (w3 . h2^2)),  r = W2 w3  (exact, prologue)
               dVdx = W1 g1
  The constant split shrinks the fp8 quantization error ~3x: the moving
  operand w3.h2^2 has much less energy than w3.(1-h2^2).
  final stage: batch-major pendulum ODE + penalties + partial sums for the
  scalar custom_loss (combined on host: pure data-parallel mean).
"""
import numpy as np
import concourse.bass as bass
import concourse.tile as tile
from concourse import bacc, mybir
from concourse.bass_utils import run_bass_kernel_spmd
from concourse.masks import make_identity

F32 = mybir.dt.float32
BF16 = mybir.dt.bfloat16
F8E4 = mybir.dt.float8e4
AF = mybir.ActivationFunctionType
ALU = mybir.AluOpType
DR = mybir.MatmulPerfMode.DoubleRow

# problem constants (hardcoded from the reference)
G = 9.8
L, I_, MB, MC, AT, AR = 0.3, 2.0, 1.0, 3.0, 0.2, 0.2
C1 = L * MB            # 0.3
C2 = I_ + L * L * MB   # 2.09
C3 = MB + MC           # 4.0
PEN = 10000.0
ALPHA = 0.1
EPS = 1e-7
C1SQ = C1 * C1
C2C3 = C2 * C3

B, H, D = 65536, 2048, 4
NCORES = 8
BC = B // NCORES        # 8192 samples per core
N = 512                 # batch-chunk (moving free dim)
CH = BC // N            # 16 chunks
KT = H // 128           # 16 feature tiles
FB = BC // 128          # 64 samples per partition in the final stage

SW = 64.0               # fp8 scale for W2^T
SG = 64.0               # fp8 scale for the moving operand w3.h2^2
INV = 1.0 / (SW * SG)

# fp32 round-to-nearest-int trick + Cody-Waite 2pi for sin/cos range reduction
RC = float(1.5 * 2 ** 23)
INV2PI = float(1.0 / (2.0 * np.pi))
TWOPI_HI = float(np.float32(2.0 * np.pi))
TWOPI_LO = float(2.0 * np.pi - np.float64(np.float32(2.0 * np.pi)))
HALFPI = float(np.pi / 2)

_NC_CACHE = {}


def build():
    nc = bacc.Bacc("TRN2", target_bir_lowering=False, debug=False)

    xd = nc.declare_dram_parameter("x", [BC, D], F32, isOutput=False)
    yd = nc.declare_dram_parameter("y", [BC], F32, isOutput=False)
    W1d = nc.declare_dram_parameter("W1", [D, H], F32, isOutput=False)
    b1d = nc.declare_dram_parameter("b1", [H], F32, isOutput=False)
    W2d = nc.declare_dram_parameter("W2", [H, H], F32, isOutput=False)
    b2d = nc.declare_dram_parameter("b2", [H], F32, isOutput=False)
    W3d = nc.declare_dram_parameter("W3", [H, 2], F32, isOutput=False)
    b3d = nc.declare_dram_parameter("b3", [2], F32, isOutput=False)

    loss_out = nc.declare_dram_parameter("loss_pen", [BC], F32, isOutput=True)
    part_out = nc.declare_dram_parameter("partials", [128, 2], F32, isOutput=True)

    with tile.TileContext(nc) as tc:
        with tc.tile_pool(name="dram", bufs=1, space="DRAM") as dpool:
            yv_d = dpool.tile([2, BC], F32, tag="yv_d", name="yv_d")
            dv_d = dpool.tile([D, BC], F32, tag="dv_d", name="dv_d")

            with tc.tile_pool(name="wpool", bufs=1) as wpool, \
                 tc.tile_pool(name="small", bufs=1) as small:

                # ---- persistent big tiles ----
                w2f = wpool.tile([128, KT, H], BF16, tag="w2f", name="w2f")
                w2b8 = wpool.tile([128, KT, H], F8E4, tag="w2b8", name="w2b8")
                h1 = wpool.tile([128, KT, N], BF16, tag="h1", name="h1")
                u1 = wpool.tile([128, KT, N], BF16, tag="u1", name="u1")
                g2m8 = wpool.tile([128, KT, N], F8E4, tag="g2m8", name="g2m8")

                # ---- small weights / constants ----
                w1sb = small.tile([D, H], BF16, tag="w1sb", name="w1sb")
                w1t = small.tile([128, KT, D], BF16, tag="w1t", name="w1t")
                w3sb = small.tile([128, KT, 2], BF16, tag="w3sb", name="w3sb")
                identb = small.tile([128, 128], BF16, tag="identb", name="identb")

                with tc.tile_pool(name="cvt", bufs=1) as cvt:
                    w1f = cvt.tile([D, H], F32, tag="w1f", name="w1f")
                    nc.sync.dma_start(w1f[:], W1d[:, :])
                    nc.vector.tensor_copy(w1sb[:], w1f[:])

                    w1tf = cvt.tile([128, KT, D], F32, tag="w1tf", name="w1tf")
                    for k in range(KT):
                        nc.gpsimd.dma_start(
                            w1tf[:, k],
                            W1d[:, k * 128:(k + 1) * 128].rearrange("d p -> p d"))
                    nc.vector.tensor_copy(w1t[:], w1tf[:])

                    w3f = cvt.tile([128, KT, 2], F32, tag="w3f", name="w3f")
                    nc.gpsimd.dma_start(w3f[:], W3d.rearrange("(k p) j -> p k j", p=128))
                    nc.vector.tensor_copy(w3sb[:], w3f[:])

                    ident_f = cvt.tile([128, 128], F32, tag="ident_f", name="ident_f")
                    make_identity(nc, ident_f[:])
                    nc.vector.tensor_copy(identb[:], ident_f[:])

                b1c = small.tile([128, KT], F32, tag="b1c", name="b1c")
                nc.gpsimd.dma_start(b1c[:], b1d.rearrange("(k p) -> p k", p=128))
                b2c = small.tile([128, KT], F32, tag="b2c", name="b2c")
                nc.gpsimd.dma_start(b2c[:], b2d.rearrange("(k p) -> p k", p=128))
                b3c = small.tile([2, 1], F32, tag="b3c", name="b3c")
                nc.gpsimd.dma_start(b3c[:], b3d.rearrange("(p o) -> p o", o=1))
                w3c1 = small.tile([128, KT, 1], F32, tag="w3c1", name="w3c1")
                nc.gpsimd.dma_start(
                    w3c1[:], W3d.rearrange("(k p) j -> p k j", p=128)[:, :, 1:2])
                # -SG * w3[:,1] per-partition scalars for the fp8 moving operand
                negw3s = small.tile([128, KT], F32, tag="negw3s", name="negw3s")
                nc.vector.tensor_scalar_mul(negw3s[:], w3c1[:, :, 0], -SG)
                # rS = SG*SW * (W2 @ w3col), accumulated during the W2 load
                rS = small.tile([128, KT], F32, tag="rS", name="rS")

                with tc.tile_pool(name="tmp", bufs=2) as tmp, \
                     tc.tile_pool(name="pp", bufs=2, space="PSUM") as pp:

                    # ---- W2 load + convert + transpose (prologue) ----
                    HQ = H // 4
                    with tc.tile_pool(name="w2cv", bufs=2) as cvp:
                        # w3 broadcast across partitions for the r matvec
                        w3row = cvp.tile([1, H], F32, tag="w3row", name="w3row",
                                         bufs=1)
                        nc.sync.dma_start(
                            w3row[:], W3d[:, 1:2].rearrange("h o -> o h"))
                        w3bc = cvp.tile([128, H], BF16, tag="w3bc", name="w3bc",
                                        bufs=1)
                        w3bcf = cvp.tile([128, H], F32, tag="w3bcf", name="w3bcf",
                                         bufs=1)
                        nc.gpsimd.partition_broadcast(w3bcf[:], w3row[:])
                        nc.vector.tensor_copy(w3bc[:], w3bcf[:])
                        r4 = cvp.tile([128, KT, 4], F32, tag="r4", name="r4",
                                      bufs=1)

                        # column-quarter-major: chunk 0's first fwd groups only
                        # need the low column blocks, so they start ~1/4 in
                        for hh in range(4):
                            hq = slice(hh * HQ, (hh + 1) * HQ)
                            for k in range(KT):
                                t = cvp.tile([128, HQ], F32, tag="w2tmp",
                                             name="w2tmp")
                                nc.sync.dma_start(
                                    t[:], W2d[k * 128:(k + 1) * 128, hq])
                                nc.vector.tensor_copy(w2f[:, k, hq], t[:])
                                # r partial: sum_j W2[128k+p, j] * w3[j]
                                prod = cvp.tile([128, HQ], F32, tag="prod",
                                                name="prod")
                                nc.gpsimd.tensor_mul(prod[:], w2f[:, k, hq],
                                                     w3bc[:, hq])
                                junk = cvp.tile([128, HQ], BF16, tag="junk",
                                                name="junk")
                                nc.scalar.activation(
                                    junk[:], prod[:], AF.Copy, bias=0.0,
                                    accum_out=r4[:, k, hh:hh + 1])
                                # W2^T tiles (fp8, scaled): for this quarter
                                for m in range(4 * hh, 4 * hh + 4):
                                    trp = pp.tile([128, 128], BF16, tag="gps",
                                                  name="trp", bufs=2)
                                    nc.tensor.transpose(
                                        trp[:], w2f[:, k, m * 128:(m + 1) * 128],
                                        identb[:])
                                    nc.vector.tensor_scalar_mul(
                                        w2b8[:, m, k * 128:(k + 1) * 128],
                                        trp[:], SW)

                        # rS = SG*SW * sum_hh r4
                        ra = cvp.tile([128, KT], F32, tag="ra", name="ra", bufs=1)
                        rb = cvp.tile([128, KT], F32, tag="rb", name="rb", bufs=1)
                        nc.vector.tensor_add(ra[:], r4[:, :, 0], r4[:, :, 1])
                        nc.vector.tensor_add(rb[:], r4[:, :, 2], r4[:, :, 3])
                        nc.vector.tensor_add(ra[:], ra[:], rb[:])
                        nc.vector.tensor_scalar_mul(rS[:], ra[:], SG * SW)

                    # ---- prologue: x chunk 0 + h1/u1 chunk 0 ----
                    def load_x(i):
                        xtf = tmp.tile([D, N], F32, tag="xtf", name="xtf", bufs=1)
                        nc.gpsimd.dma_start(
                            xtf[:], xd[i * N:(i + 1) * N, :].rearrange("n d -> d n"))
                        xt = tmp.tile([D, N], BF16, tag="xt", name="xt")
                        nc.vector.tensor_copy(xt[:], xtf[:])
                        return xt

                    def fwd1_mm(m1, xt):
                        # h1 for feature block m1 from xt (chunk's transposed x)
                        hps = pp.tile([128, N], F32, tag="h1ps", name="hps", bufs=2)
                        nc.tensor.matmul(hps[:], w1sb[:, m1 * 128:(m1 + 1) * 128],
                                         xt[:], start=True, stop=True)
                        nc.scalar.activation(h1[:, m1], hps[:], AF.Tanh,
                                             bias=b1c[:, m1:m1 + 1])

                    def u1_ops(m1):
                        # u1 = 1 - h1^2 (on Pool: off the ACT/DVE critical path)
                        nc.gpsimd.tensor_mul(u1[:, m1], h1[:, m1], h1[:, m1])
                        nc.gpsimd.tensor_scalar(u1[:, m1], u1[:, m1], -1.0, 1.0,
                                                ALU.mult, ALU.add)

                    xt_cur = load_x(0)
                    for m1 in range(KT):
                        fwd1_mm(m1, xt_cur)
                        u1_ops(m1)

                    # ---- main loop over chunks ----
                    for i in range(CH):
                        # ---- phase A: fwd W2 / W3 / g2m ----
                        yvp = pp.tile([2, N], F32, tag="yvdv", name="yvp", bufs=1)
                        h2ts = {}
                        for m2 in range(KT):
                            ps = pp.tile([128, N], F32, tag="ps", name="ps", bufs=3)
                            for k in range(KT):
                                nc.tensor.matmul(
                                    ps[:], w2f[:, k, m2 * 128:(m2 + 1) * 128],
                                    h1[:, k], start=(k == 0), stop=(k == KT - 1))
                            # W3 output accumulation trails by 2 groups so the
                            # tanh producer is never on the PE critical path
                            if m2 > 1:
                                nc.tensor.matmul(yvp[:], w3sb[:, m2 - 2],
                                                 h2ts[m2 - 2][:],
                                                 start=(m2 == 2), stop=False)
                            h2t = tmp.tile([128, N], BF16, tag="h2t", name="h2t",
                                           bufs=4)
                            nc.scalar.activation(h2t[:], ps[:], AF.Tanh,
                                                 bias=b2c[:, m2:m2 + 1])
                            sq = tmp.tile([128, N], BF16, tag="sq", name="sq",
                                          bufs=2)
                            nc.gpsimd.tensor_mul(sq[:], h2t[:], h2t[:])
                            # moving operand: -SG * w3 * h2^2 quantized to fp8
                            nc.vector.tensor_scalar(
                                g2m8[:, m2], sq[:], negw3s[:, m2:m2 + 1], None,
                                ALU.mult)
                            h2ts[m2] = h2t
                        nc.tensor.matmul(yvp[:], w3sb[:, KT - 2], h2ts[KT - 2][:],
                                         start=False, stop=False)
                        nc.tensor.matmul(yvp[:], w3sb[:, KT - 1], h2ts[KT - 1][:],
                                         start=False, stop=True)
                        yvt = tmp.tile([2, N], F32, tag="yvt", name="yvt", bufs=1)
                        nc.vector.tensor_scalar(yvt[:], yvp[:], b3c[:], None,
                                                ALU.add)
                        nc.sync.dma_start(yv_d[:, i * N:(i + 1) * N], yvt[:])

                        # x for next chunk
                        if i + 1 < CH:
                            xt_cur = load_x(i + 1)

                        # ---- phase C: bwd fp8 DoubleRow / dVdx (+ fwd W1 of
                        # chunk i+1) ----
                        dvp = pp.tile([D, N], F32, tag="yvdv", name="dvp", bufs=1)
                        g1h_prev = None
                        NP = KT // 2
                        for m1 in range(KT):
                            gps = pp.tile([128, N], F32, tag="gps", name="gps", bufs=2)
                            torder = (list(range(NP)) if m1 < KT - 1
                                      else list(range(NP - 1, -1, -1)))
                            for j, t2 in enumerate(torder):
                                nc.tensor.matmul(
                                    gps[:],
                                    w2b8[:, 2 * t2:2 * t2 + 2,
                                         m1 * 128:(m1 + 1) * 128],
                                    g2m8[:, 2 * t2:2 * t2 + 2, :],
                                    start=(j == 0), stop=(j == NP - 1),
                                    perf_mode=DR)
                            if m1 > 0:
                                nc.tensor.matmul(dvp[:], w1t[:, m1 - 1],
                                                 g1h_prev[:],
                                                 start=(m1 == 1), stop=False)
                            # interleaved fwd W1+tanh for chunk i+1, hoisted 2
                            # iterations early so the h1 tanh chain never backs
                            # up the ACT queue into the next chunk's h2t tanh.
                            # h1[m1] is safe to overwrite once chunk i's phase A
                            # is done; only u1[m1] must wait for g1h(m1) below.
                            if i + 1 < CH:
                                if m1 == 0:
                                    fwd1_mm(0, xt_cur)
                                    fwd1_mm(1, xt_cur)
                                if m1 + 2 < KT:
                                    fwd1_mm(m1 + 2, xt_cur)
                            # g1 = u1 * (r + psum/(SG*SW))
                            gt = tmp.tile([128, N], F32, tag="gt", name="gt",
                                          bufs=2)
                            nc.vector.tensor_scalar(gt[:], gps[:],
                                                    rS[:, m1:m1 + 1], INV,
                                                    ALU.add, ALU.mult)
                            g1h = tmp.tile([128, N], BF16, tag="g1h", name="g1h",
                                           bufs=2)
                            nc.vector.tensor_mul(g1h[:], gt[:], u1[:, m1])
                            if i + 1 < CH:
                                u1_ops(m1)
                            g1h_prev = g1h
                        nc.tensor.matmul(dvp[:], w1t[:, KT - 1], g1h_prev[:],
                                         start=False, stop=True)
                        dvt = tmp.tile([D, N], F32, tag="dvt", name="dvt", bufs=1)
                        nc.vector.tensor_copy(dvt[:], dvp[:])
                        nc.sync.dma_start(dv_d[:, i * N:(i + 1) * N], dvt[:])

                # ---- final stage: batch-major per-sample math ----
                with tc.tile_pool(name="fpool", bufs=1) as fpool:
                    _final_stage(nc, tc, fpool, xd, yd, yv_d, dv_d,
                                 loss_out, part_out)

    nc.compile()
    return nc


def _final_stage(nc, tc, fpool, xd, yd, yv_d, dv_d, loss_out, part_out):
    def plane_from_row(dram_row_ap, tag):
        t = fpool.tile([128, FB], F32, tag=tag, name=tag)
        nc.sync.dma_start(t[:], dram_row_ap.rearrange("(p f) -> p f", p=128))
        return t

    ypred = plane_from_row(yv_d[0], "ypred")
    vpl = plane_from_row(yv_d[1], "vpl")
    dv0 = plane_from_row(dv_d[0], "dv0")
    dv1 = plane_from_row(dv_d[1], "dv1")
    dv2 = plane_from_row(dv_d[2], "dv2")
    dv3 = plane_from_row(dv_d[3], "dv3")
    ypl = plane_from_row(yd[:], "ypl")

    xpl = fpool.tile([128, FB, D], F32, tag="xpl", name="xpl")
    nc.sync.dma_start(xpl[:], xd.rearrange("(p f) d -> p f d", p=128))
    x2 = xpl[:, :, 1]
    x3 = xpl[:, :, 2]
    x4 = xpl[:, :, 3]

    zc = fpool.tile([128, 1], F32, tag="zc", name="zc")
    nc.vector.memset(zc[:], 0.0)

    def ftile(tag):
        return fpool.tile([128, FB], F32, tag=tag, name=tag)

    def sin_reduced(src_ap, negate, bias, tag):
        # sin(bias + (negate ? -src : src)), range-reduced mod 2pi
        w = ftile(tag + "w")
        nc.vector.tensor_scalar(w[:], src_ap, -1.0 if negate else 1.0, bias,
                                ALU.mult, ALU.add)
        t = ftile(tag + "t")
        nc.vector.tensor_scalar(t[:], w[:], INV2PI, RC, ALU.mult, ALU.add)
        r = ftile(tag + "r")
        nc.vector.tensor_scalar(r[:], t[:], RC, None, ALU.subtract)
        a = ftile(tag + "a")
        nc.vector.scalar_tensor_tensor(a[:], r[:], -TWOPI_HI, w[:], ALU.mult, ALU.add)
        y_ = ftile(tag + "y")
        nc.vector.scalar_tensor_tensor(y_[:], r[:], -TWOPI_LO, a[:], ALU.mult, ALU.add)
        o = ftile(tag + "o")
        nc.scalar.activation(o[:], y_[:], AF.Sin, bias=zc[:])
        return o

    s = sin_reduced(x3, False, 0.0, "s")
    c = sin_reduced(x3, True, HALFPI, "c")     # cos(x) = sin(pi/2 - x)

    f = ftile("f")
    nc.vector.scalar_tensor_tensor(f[:], x2, -AT, ypred[:], ALU.mult, ALU.add)

    u = ftile("u")
    nc.vector.tensor_mul(u[:], c[:], c[:])
    den = ftile("den")
    nc.vector.tensor_scalar(den[:], u[:], -C1SQ, C2C3, ALU.mult, ALU.add)
    rden = ftile("rden")
    nc.vector.reciprocal(rden[:], den[:])

    cs = ftile("cs")
    nc.vector.tensor_mul(cs[:], c[:], s[:])
    x4sq = ftile("x4sq")
    nc.vector.tensor_mul(x4sq[:], x4, x4)
    cx4 = ftile("cx4")
    nc.vector.tensor_mul(cx4[:], c[:], x4)
    sx4sq = ftile("sx4sq")
    nc.vector.tensor_mul(sx4sq[:], s[:], x4sq[:])
    csx4sq = ftile("csx4sq")
    nc.vector.tensor_mul(csx4sq[:], cs[:], x4sq[:])
    cf = ftile("cf")
    nc.vector.tensor_mul(cf[:], c[:], f[:])

    # x2p = (G*C1^2*c*s + C2*f - AR*C1*c*x4 - C1*C2*s*x4^2) / den
    p1 = ftile("p1")
    nc.vector.tensor_scalar(p1[:], f[:], C2, None, ALU.mult)
    nc.vector.scalar_tensor_tensor(p1[:], cs[:], G * C1SQ, p1[:], ALU.mult, ALU.add)
    nc.vector.scalar_tensor_tensor(p1[:], cx4[:], -AR * C1, p1[:], ALU.mult, ALU.add)
    nc.vector.scalar_tensor_tensor(p1[:], sx4sq[:], -C1 * C2, p1[:], ALU.mult, ALU.add)
    x2p = ftile("x2p")
    nc.vector.tensor_mul(x2p[:], p1[:], rden[:])

    # x4p = (G*C1*C3*s + C1*c*f - AR*C3*x4 - C1^2*c*s*x4^2) / den
    p2 = ftile("p2")
    nc.vector.tensor_scalar(p2[:], s[:], G * C1 * C3, None, ALU.mult)
    nc.vector.scalar_tensor_tensor(p2[:], cf[:], C1, p2[:], ALU.mult, ALU.add)
    nc.vector.scalar_tensor_tensor(p2[:], x4, -AR * C3, p2[:], ALU.mult, ALU.add)
    nc.vector.scalar_tensor_tensor(p2[:], csx4sq[:], -C1SQ, p2[:], ALU.mult, ALU.add)
    x4p = ftile("x4p")
    nc.vector.tensor_mul(x4p[:], p2[:], rden[:])

    # Vdot = dV . [x2, x2p, x4, x4p]
    vd = ftile("vd")
    nc.vector.tensor_mul(vd[:], dv0[:], x2)
    t_ = ftile("vt")
    nc.vector.tensor_mul(t_[:], dv1[:], x2p[:])
    nc.vector.tensor_add(vd[:], vd[:], t_[:])
    nc.vector.tensor_mul(t_[:], dv2[:], x4)
    nc.vector.tensor_add(vd[:], vd[:], t_[:])
    nc.vector.tensor_mul(t_[:], dv3[:], x4p[:])
    nc.vector.tensor_add(vd[:], vd[:], t_[:])

    # penalties: PEN*relu(-V) + PEN*relu(Vdot)
    pen = ftile("pen")
    nc.vector.tensor_scalar(pen[:], vpl[:], 0.0, -PEN, ALU.min, ALU.mult)
    pen2 = ftile("pen2")
    nc.vector.tensor_scalar(pen2[:], vd[:], 0.0, PEN, ALU.max, ALU.mult)
    nc.vector.tensor_add(pen[:], pen[:], pen2[:])
    nc.sync.dma_start(loss_out.rearrange("(p f) -> p f", p=128), pen[:])

    # partial sums for custom_loss: sum(d^2), sum((y - y_pred)^2)
    ypc = ftile("ypc")
    nc.vector.tensor_scalar(ypc[:], ypred[:], EPS, None, ALU.max)
    l1 = ftile("l1")
    nc.scalar.activation(l1[:], ypc[:], AF.Ln, bias=1.0)
    yc = ftile("yc")
    nc.vector.tensor_scalar(yc[:], ypl[:], EPS, None, ALU.max)
    l2 = ftile("l2")
    nc.scalar.activation(l2[:], yc[:], AF.Ln, bias=1.0)
    dd = ftile("dd")
    nc.vector.tensor_sub(dd[:], l1[:], l2[:])
    d2s = fpool.tile([128, 1], F32, tag="d2s", name="d2s")
    dtmp = ftile("dtmp")
    nc.scalar.activation(dtmp[:], dd[:], AF.Square, bias=zc[:], accum_out=d2s[:])
    ee = ftile("ee")
    nc.vector.tensor_sub(ee[:], ypl[:], ypred[:])
    es = fpool.tile([128, 1], F32, tag="es", name="es")
    nc.scalar.activation(dtmp[:], ee[:], AF.Square, bias=zc[:], accum_out=es[:])

    parts = fpool.tile([128, 2], F32, tag="parts", name="parts")
    nc.vector.tensor_copy(parts[:, 0:1], d2s[:])
    nc.vector.tensor_copy(parts[:, 1:2], es[:])
    nc.sync.dma_start(part_out[:, :], parts[:])


def kernel(**inputs):
    x = np.ascontiguousarray(inputs["x"], dtype=np.float32)
    y = np.ascontiguousarray(inputs["y"], dtype=np.float32)
    W1 = np.ascontiguousarray(inputs["W1"], dtype=np.float32)
    b1 = np.ascontiguousarray(inputs["b1"], dtype=np.float32)
    W2 = np.ascontiguousarray(inputs["W2"], dtype=np.float32)
    b2 = np.ascontiguousarray(inputs["b2"], dtype=np.float32)
    W3 = np.ascontiguousarray(inputs["W3"], dtype=np.float32)
    b3 = np.ascontiguousarray(inputs["b3"], dtype=np.float32)

    if "nc" not in _NC_CACHE:
        _NC_CACHE["nc"] = build()
    nc = _NC_CACHE["nc"]

    in_maps = []
    for cid in range(NCORES):
        sl = slice(cid * BC, (cid + 1) * BC)
        in_maps.append({
            "x": x[sl], "y": y[sl],
            "W1": W1, "b1": b1, "W2": W2, "b2": b2, "W3": W3, "b3": b3,
        })
    res = run_bass_kernel_spmd(nc, in_maps, list(range(NCORES)))

    loss = np.concatenate([res.results[c]["loss_pen"] for c in range(NCORES)])
    parts = np.stack([res.results[c]["partials"] for c in range(NCORES)])
    sums = parts.astype(np.float64).sum(axis=(0, 1))
    scalar = ALPHA * sums[0] / B + (1.0 - ALPHA) * sums[1] / B
    return (loss + np.float32(scalar)).astype(np.float32)


# revision 13
# speedup vs baseline: 3.3574x; 2.8571x over previous
"""Trainium2 Bass kernel for nn_CustomModel_21019569946955 (pendulum Lyapunov loss).

Data-parallel over 8 NeuronCores: each core processes B/8 = 8192 samples with
replicated MLP weights. Fused single-pass pipeline:

  fwd (bf16):  h1 = tanh(W1^T x^T); u1 = 1-h1^2
               h2 = tanh(W2^T h1); [y_pred; V] = W3^T h2
  bwd (fp8 DoubleRow + constant split):
               g1 = u1 * (r - W2 (w3 . h2^2)),  r = W2 w3  (exact, prologue)
               dVdx = W1 g1
  The constant split shrinks the fp8 quantization error ~3x: the moving
  operand w3.h2^2 has much less energy than w3.(1-h2^2).
  final stage: batch-major pendulum ODE + penalties + partial sums for the
  scalar custom_loss (combined on host: pure data-parallel mean).
"""
import numpy as np
import concourse.bass as bass
import concourse.tile as tile
from concourse import bacc, mybir
from concourse.bass_utils import run_bass_kernel_spmd
from concourse.masks import make_identity

F32 = mybir.dt.float32
BF16 = mybir.dt.bfloat16
F8E4 = mybir.dt.float8e4
AF = mybir.ActivationFunctionType
ALU = mybir.AluOpType
DR = mybir.MatmulPerfMode.DoubleRow

# problem constants (hardcoded from the reference)
G = 9.8
L, I_, MB, MC, AT, AR = 0.3, 2.0, 1.0, 3.0, 0.2, 0.2
C1 = L * MB            # 0.3
C2 = I_ + L * L * MB   # 2.09
C3 = MB + MC           # 4.0
PEN = 10000.0
ALPHA = 0.1
EPS = 1e-7
C1SQ = C1 * C1
C2C3 = C2 * C3

import os
B, H, D = 65536, 2048, 4
NCORES = 8
BC = B // NCORES        # 8192 samples per core
N = 512                 # batch-chunk (moving free dim)
CH = BC // N            # 16 chunks
CH_RUN = int(os.environ.get("KCHUNKS", CH))  # timing experiments only
KT = H // 128           # 16 feature tiles
FB = BC // 128          # 64 samples per partition in the final stage

SW = 64.0               # fp8 scale for W2^T
SG = 64.0               # fp8 scale for the moving operand w3.h2^2
INV = 1.0 / (SW * SG)

# fp32 round-to-nearest-int trick + Cody-Waite 2pi for sin/cos range reduction
RC = float(1.5 * 2 ** 23)
INV2PI = float(1.0 / (2.0 * np.pi))
TWOPI_HI = float(np.float32(2.0 * np.pi))
TWOPI_LO = float(2.0 * np.pi - np.float64(np.float32(2.0 * np.pi)))
HALFPI = float(np.pi / 2)

_NC_CACHE = {}


def build():
    nc = bacc.Bacc("TRN2", target_bir_lowering=False, debug=False)

    xd = nc.declare_dram_parameter("x", [BC, D], F32, isOutput=False)
    yd = nc.declare_dram_parameter("y", [BC], F32, isOutput=False)
    W1d = nc.declare_dram_parameter("W1", [D, H], F32, isOutput=False)
    b1d = nc.declare_dram_parameter("b1", [H], F32, isOutput=False)
    W2d = nc.declare_dram_parameter("W2", [H, H], F32, isOutput=False)
    b2d = nc.declare_dram_parameter("b2", [H], F32, isOutput=False)
    W3d = nc.declare_dram_parameter("W3", [H, 2], F32, isOutput=False)
    b3d = nc.declare_dram_parameter("b3", [2], F32, isOutput=False)

    loss_out = nc.declare_dram_parameter("loss_pen", [BC], F32, isOutput=True)
    part_out = nc.declare_dram_parameter("partials", [128, 4], F32, isOutput=True)

    with tile.TileContext(nc) as tc:
        with tc.tile_pool(name="dram", bufs=1, space="DRAM") as dpool:
            yv_d = dpool.tile([2, BC], F32, tag="yv_d", name="yv_d")
            dv_d = dpool.tile([D, BC], F32, tag="dv_d", name="dv_d")

            with tc.tile_pool(name="wpool", bufs=1) as wpool, \
                 tc.tile_pool(name="small", bufs=1) as small:

                # ---- persistent big tiles ----
                w2f = wpool.tile([128, KT, H], BF16, tag="w2f", name="w2f")
                w2b8 = wpool.tile([128, KT, H], F8E4, tag="w2b8", name="w2b8")
                h1 = wpool.tile([128, KT, N], BF16, tag="h1", name="h1")
                u1 = wpool.tile([128, KT, N], BF16, tag="u1", name="u1")
                g2m8 = wpool.tile([128, KT, N], F8E4, tag="g2m8", name="g2m8")

                # ---- small weights / constants ----
                w1sb = small.tile([D, H], BF16, tag="w1sb", name="w1sb")
                w1t = small.tile([128, KT, D], BF16, tag="w1t", name="w1t")
                w3sb = small.tile([128, KT, 2], BF16, tag="w3sb", name="w3sb")
                identb = small.tile([128, 128], BF16, tag="identb", name="identb")

                with tc.tile_pool(name="cvt", bufs=1) as cvt:
                    w1f = cvt.tile([D, H], F32, tag="w1f", name="w1f")
                    nc.sync.dma_start(w1f[:], W1d[:, :])
                    nc.vector.tensor_copy(w1sb[:], w1f[:])

                    w1tf = cvt.tile([128, KT, D], F32, tag="w1tf", name="w1tf")
                    for k in range(KT):
                        nc.gpsimd.dma_start(
                            w1tf[:, k],
                            W1d[:, k * 128:(k + 1) * 128].rearrange("d p -> p d"))
                    nc.vector.tensor_copy(w1t[:], w1tf[:])

                    w3f = cvt.tile([128, KT, 2], F32, tag="w3f", name="w3f")
                    nc.gpsimd.dma_start(w3f[:], W3d.rearrange("(k p) j -> p k j", p=128))
                    nc.vector.tensor_copy(w3sb[:], w3f[:])

                    ident_f = cvt.tile([128, 128], F32, tag="ident_f", name="ident_f")
                    make_identity(nc, ident_f[:])
                    nc.vector.tensor_copy(identb[:], ident_f[:])

                b1c = small.tile([128, KT], F32, tag="b1c", name="b1c")
                nc.gpsimd.dma_start(b1c[:], b1d.rearrange("(k p) -> p k", p=128))
                b2c = small.tile([128, KT], F32, tag="b2c", name="b2c")
                nc.gpsimd.dma_start(b2c[:], b2d.rearrange("(k p) -> p k", p=128))
                b3c = small.tile([2, 1], F32, tag="b3c", name="b3c")
                nc.gpsimd.dma_start(b3c[:], b3d.rearrange("(p o) -> p o", o=1))
                w3c1 = small.tile([128, KT, 1], F32, tag="w3c1", name="w3c1")
                nc.gpsimd.dma_start(
                    w3c1[:], W3d.rearrange("(k p) j -> p k j", p=128)[:, :, 1:2])
                # -SG * w3[:,1] per-partition scalars for the fp8 moving operand
                negw3s = small.tile([128, KT], F32, tag="negw3s", name="negw3s")
                nc.vector.tensor_scalar_mul(negw3s[:], w3c1[:, :, 0], -SG)
                # rS = SG*SW * (W2 @ w3col), accumulated during the W2 load
                rS = small.tile([128, KT], F32, tag="rS", name="rS")

                with tc.tile_pool(name="tmp", bufs=2) as tmp, \
                     tc.tile_pool(name="pp", bufs=2, space="PSUM") as pp, \
                     tc.tile_pool(name="fpool", bufs=1) as fpool:

                    # ---- W2 load + convert + transpose (prologue) ----
                    HQ = H // 4
                    with tc.tile_pool(name="w2cv", bufs=2) as cvp:
                        # w3 broadcast across partitions for the r matvec
                        w3row = cvp.tile([1, H], F32, tag="w3row", name="w3row",
                                         bufs=1)
                        nc.sync.dma_start(
                            w3row[:], W3d[:, 1:2].rearrange("h o -> o h"))
                        w3bc = cvp.tile([128, H], BF16, tag="w3bc", name="w3bc",
                                        bufs=1)
                        w3bcf = cvp.tile([128, H], F32, tag="w3bcf", name="w3bcf",
                                         bufs=1)
                        nc.gpsimd.partition_broadcast(w3bcf[:], w3row[:])
                        nc.vector.tensor_copy(w3bc[:], w3bcf[:])
                        r4 = cvp.tile([128, KT, 4], F32, tag="r4", name="r4",
                                      bufs=1)

                        # column-quarter-major: chunk 0's first fwd groups only
                        # need the low column blocks, so they start ~1/4 in
                        for hh in range(4):
                            hq = slice(hh * HQ, (hh + 1) * HQ)
                            for k in range(KT):
                                t = cvp.tile([128, HQ], F32, tag="w2tmp",
                                             name="w2tmp")
                                nc.sync.dma_start(
                                    t[:], W2d[k * 128:(k + 1) * 128, hq])
                                nc.vector.tensor_copy(w2f[:, k, hq], t[:])
                                # r partial: sum_j W2[128k+p, j] * w3[j]
                                prod = cvp.tile([128, HQ], F32, tag="prod",
                                                name="prod")
                                nc.gpsimd.tensor_mul(prod[:], w2f[:, k, hq],
                                                     w3bc[:, hq])
                                junk = cvp.tile([128, HQ], BF16, tag="junk",
                                                name="junk")
                                nc.scalar.activation(
                                    junk[:], prod[:], AF.Copy, bias=0.0,
                                    accum_out=r4[:, k, hh:hh + 1])
                                # W2^T tiles (fp8, scaled): for this quarter
                                for m in range(4 * hh, 4 * hh + 4):
                                    trp = pp.tile([128, 128], BF16, tag="gps",
                                                  name="trp", bufs=2)
                                    nc.tensor.transpose(
                                        trp[:], w2f[:, k, m * 128:(m + 1) * 128],
                                        identb[:])
                                    nc.vector.tensor_scalar_mul(
                                        w2b8[:, m, k * 128:(k + 1) * 128],
                                        trp[:], SW)

                        # rS = SG*SW * sum_hh r4
                        ra = cvp.tile([128, KT], F32, tag="ra", name="ra", bufs=1)
                        rb = cvp.tile([128, KT], F32, tag="rb", name="rb", bufs=1)
                        nc.vector.tensor_add(ra[:], r4[:, :, 0], r4[:, :, 1])
                        nc.vector.tensor_add(rb[:], r4[:, :, 2], r4[:, :, 3])
                        nc.vector.tensor_add(ra[:], ra[:], rb[:])
                        nc.vector.tensor_scalar_mul(rS[:], ra[:], SG * SW)

                    # ---- prologue: x chunk 0 + h1/u1 chunk 0 ----
                    def load_x(i):
                        xtf = tmp.tile([D, N], F32, tag="xtf", name="xtf", bufs=1)
                        nc.gpsimd.dma_start(
                            xtf[:], xd[i * N:(i + 1) * N, :].rearrange("n d -> d n"))
                        xt = tmp.tile([D, N], BF16, tag="xt", name="xt")
                        nc.vector.tensor_copy(xt[:], xtf[:])
                        return xt

                    def fwd1_mm(m1, xt):
                        # h1 for feature block m1 from xt (chunk's transposed x)
                        hps = pp.tile([128, N], F32, tag="h1ps", name="hps", bufs=2)
                        nc.tensor.matmul(hps[:], w1sb[:, m1 * 128:(m1 + 1) * 128],
                                         xt[:], start=True, stop=True)
                        nc.scalar.activation(h1[:, m1], hps[:], AF.Tanh,
                                             bias=b1c[:, m1:m1 + 1])

                    def u1_ops(m1):
                        # u1 = 1 - h1^2 (on Pool: off the ACT/DVE critical path)
                        nc.gpsimd.tensor_mul(u1[:, m1], h1[:, m1], h1[:, m1])
                        nc.gpsimd.tensor_scalar(u1[:, m1], u1[:, m1], -1.0, 1.0,
                                                ALU.mult, ALU.add)

                    xt_cur = load_x(0)
                    for m1 in range(KT):
                        fwd1_mm(m1, xt_cur)
                        u1_ops(m1)

                    # ---- main loop over chunks ----
                    for i in range(CH_RUN):
                        # ---- phase A: fwd W2 / W3 / g2m ----
                        yvp = pp.tile([2, N], F32, tag="yvdv", name="yvp", bufs=1)
                        h2ts = {}
                        for m2 in range(KT):
                            ps = pp.tile([128, N], F32, tag="ps", name="ps", bufs=3)
                            for k in range(KT):
                                nc.tensor.matmul(
                                    ps[:], w2f[:, k, m2 * 128:(m2 + 1) * 128],
                                    h1[:, k], start=(k == 0), stop=(k == KT - 1))
                            # W3 output accumulation trails by 2 groups so the
                            # tanh producer is never on the PE critical path
                            if m2 > 1:
                                nc.tensor.matmul(yvp[:], w3sb[:, m2 - 2],
                                                 h2ts[m2 - 2][:],
                                                 start=(m2 == 2), stop=False)
                            h2t = tmp.tile([128, N], BF16, tag="h2t", name="h2t",
                                           bufs=4)
                            nc.scalar.activation(h2t[:], ps[:], AF.Tanh,
                                                 bias=b2c[:, m2:m2 + 1])
                            sq = tmp.tile([128, N], BF16, tag="sq", name="sq",
                                          bufs=2)
                            nc.gpsimd.tensor_mul(sq[:], h2t[:], h2t[:])
                            # moving operand: -SG * w3 * h2^2 quantized to fp8
                            nc.vector.tensor_scalar(
                                g2m8[:, m2], sq[:], negw3s[:, m2:m2 + 1], None,
                                ALU.mult)
                            h2ts[m2] = h2t
                        nc.tensor.matmul(yvp[:], w3sb[:, KT - 2], h2ts[KT - 2][:],
                                         start=False, stop=False)
                        nc.tensor.matmul(yvp[:], w3sb[:, KT - 1], h2ts[KT - 1][:],
                                         start=False, stop=True)
                        yvt = tmp.tile([2, N], F32, tag="yvt", name="yvt", bufs=1)
                        nc.vector.tensor_scalar(yvt[:], yvp[:], b3c[:], None,
                                                ALU.add)
                        nc.sync.dma_start(yv_d[:, i * N:(i + 1) * N], yvt[:])

                        # x for next chunk
                        if i + 1 < CH_RUN:
                            xt_cur = load_x(i + 1)

                        # ---- phase C: bwd fp8 DoubleRow / dVdx (+ fwd W1 of
                        # chunk i+1) ----
                        dvp = pp.tile([D, N], F32, tag="yvdv", name="dvp", bufs=1)
                        g1h_prev = None
                        NP = KT // 2
                        for m1 in range(KT):
                            gps = pp.tile([128, N], F32, tag="gps", name="gps", bufs=2)
                            torder = (list(range(NP)) if m1 < KT - 1
                                      else list(range(NP - 1, -1, -1)))
                            for j, t2 in enumerate(torder):
                                nc.tensor.matmul(
                                    gps[:],
                                    w2b8[:, 2 * t2:2 * t2 + 2,
                                         m1 * 128:(m1 + 1) * 128],
                                    g2m8[:, 2 * t2:2 * t2 + 2, :],
                                    start=(j == 0), stop=(j == NP - 1),
                                    perf_mode=DR)
                            if m1 > 0:
                                nc.tensor.matmul(dvp[:], w1t[:, m1 - 1],
                                                 g1h_prev[:],
                                                 start=(m1 == 1), stop=False)
                            # interleaved fwd W1+tanh for chunk i+1, hoisted 2
                            # iterations early so the h1 tanh chain never backs
                            # up the ACT queue into the next chunk's h2t tanh.
                            # h1[m1] is safe to overwrite once chunk i's phase A
                            # is done; only u1[m1] must wait for g1h(m1) below.
                            if i + 1 < CH_RUN:
                                if m1 == 0:
                                    fwd1_mm(0, xt_cur)
                                    fwd1_mm(1, xt_cur)
                                if m1 + 2 < KT:
                                    fwd1_mm(m1 + 2, xt_cur)
                            # g1 = u1 * (r + psum/(SG*SW))
                            gt = tmp.tile([128, N], F32, tag="gt", name="gt",
                                          bufs=2)
                            nc.vector.tensor_scalar(gt[:], gps[:],
                                                    rS[:, m1:m1 + 1], INV,
                                                    ALU.add, ALU.mult)
                            g1h = tmp.tile([128, N], BF16, tag="g1h", name="g1h",
                                           bufs=2)
                            nc.vector.tensor_mul(g1h[:], gt[:], u1[:, m1])
                            if i + 1 < CH_RUN:
                                u1_ops(m1)
                            g1h_prev = g1h
                        nc.tensor.matmul(dvp[:], w1t[:, KT - 1], g1h_prev[:],
                                         start=False, stop=True)
                        dvt = tmp.tile([D, N], F32, tag="dvt", name="dvt", bufs=1)
                        nc.vector.tensor_copy(dvt[:], dvp[:])
                        nc.sync.dma_start(dv_d[:, i * N:(i + 1) * N], dvt[:])

                        # first-half final stage overlaps with chunks 8..15
                        if i == CH // 2 - 1 and CH_RUN == CH:
                            _final_stage(nc, tc, fpool, xd, yd, yv_d, dv_d,
                                         loss_out, part_out, half=0)

                    # ---- final stage, second half ----
                    if CH_RUN == CH:
                        _final_stage(nc, tc, fpool, xd, yd, yv_d, dv_d,
                                     loss_out, part_out, half=1)
                    else:
                        # timing-only truncated build: keep outputs written
                        zt = tmp.tile([128, FB], F32, tag="zt", name="zt",
                                      bufs=1)
                        nc.vector.memset(zt[:], 0.0)
                        nc.sync.dma_start(
                            loss_out.rearrange("(p f) -> p f", p=128), zt[:])
                        nc.sync.dma_start(part_out[:, :], zt[:, 0:4])

    nc.compile()
    return nc


def _final_stage(nc, tc, fpool, xd, yd, yv_d, dv_d, loss_out, part_out, half):
    BCH = BC // 2
    FBH = BCH // 128
    s0 = half * BCH

    def plane_from_row(dram_row_ap, tag):
        t = fpool.tile([128, FBH], F32, tag=tag, name=tag)
        nc.sync.dma_start(
            t[:], dram_row_ap[s0:s0 + BCH].rearrange("(p f) -> p f", p=128))
        return t

    ypred = plane_from_row(yv_d[0], "ypred")
    vpl = plane_from_row(yv_d[1], "vpl")
    dv0 = plane_from_row(dv_d[0], "dv0")
    dv1 = plane_from_row(dv_d[1], "dv1")
    dv2 = plane_from_row(dv_d[2], "dv2")
    dv3 = plane_from_row(dv_d[3], "dv3")
    ypl = plane_from_row(yd[:], "ypl")

    xpl = fpool.tile([128, FBH, D], F32, tag="xpl", name="xpl")
    nc.sync.dma_start(
        xpl[:], xd[s0:s0 + BCH].rearrange("(p f) d -> p f d", p=128))
    x2 = xpl[:, :, 1]
    x3 = xpl[:, :, 2]
    x4 = xpl[:, :, 3]

    zc = fpool.tile([128, 1], F32, tag="zc", name="zc")
    nc.vector.memset(zc[:], 0.0)

    def ftile(tag):
        return fpool.tile([128, FBH], F32, tag=tag, name=tag)

    def sin_reduced(src_ap, negate, bias, tag):
        # sin(bias + (negate ? -src : src)), range-reduced mod 2pi
        w = ftile(tag + "w")
        nc.vector.tensor_scalar(w[:], src_ap, -1.0 if negate else 1.0, bias,
                                ALU.mult, ALU.add)
        t = ftile(tag + "t")
        nc.vector.tensor_scalar(t[:], w[:], INV2PI, RC, ALU.mult, ALU.add)
        r = ftile(tag + "r")
        nc.vector.tensor_scalar(r[:], t[:], RC, None, ALU.subtract)
        a = ftile(tag + "a")
        nc.vector.scalar_tensor_tensor(a[:], r[:], -TWOPI_HI, w[:], ALU.mult, ALU.add)
        y_ = ftile(tag + "y")
        nc.vector.scalar_tensor_tensor(y_[:], r[:], -TWOPI_LO, a[:], ALU.mult, ALU.add)
        o = ftile(tag + "o")
        nc.scalar.activation(o[:], y_[:], AF.Sin, bias=zc[:])
        return o

    s = sin_reduced(x3, False, 0.0, "s")
    c = sin_reduced(x3, True, HALFPI, "c")     # cos(x) = sin(pi/2 - x)

    f = ftile("f")
    nc.vector.scalar_tensor_tensor(f[:], x2, -AT, ypred[:], ALU.mult, ALU.add)

    u = ftile("u")
    nc.vector.tensor_mul(u[:], c[:], c[:])
    den = ftile("den")
    nc.vector.tensor_scalar(den[:], u[:], -C1SQ, C2C3, ALU.mult, ALU.add)
    rden = ftile("rden")
    nc.vector.reciprocal(rden[:], den[:])

    cs = ftile("cs")
    nc.vector.tensor_mul(cs[:], c[:], s[:])
    x4sq = ftile("x4sq")
    nc.vector.tensor_mul(x4sq[:], x4, x4)
    cx4 = ftile("cx4")
    nc.vector.tensor_mul(cx4[:], c[:], x4)
    sx4sq = ftile("sx4sq")
    nc.vector.tensor_mul(sx4sq[:], s[:], x4sq[:])
    csx4sq = ftile("csx4sq")
    nc.vector.tensor_mul(csx4sq[:], cs[:], x4sq[:])
    cf = ftile("cf")
    nc.vector.tensor_mul(cf[:], c[:], f[:])

    # x2p = (G*C1^2*c*s + C2*f - AR*C1*c*x4 - C1*C2*s*x4^2) / den
    p1 = ftile("p1")
    nc.vector.tensor_scalar(p1[:], f[:], C2, None, ALU.mult)
    nc.vector.scalar_tensor_tensor(p1[:], cs[:], G * C1SQ, p1[:], ALU.mult, ALU.add)
    nc.vector.scalar_tensor_tensor(p1[:], cx4[:], -AR * C1, p1[:], ALU.mult, ALU.add)
    nc.vector.scalar_tensor_tensor(p1[:], sx4sq[:], -C1 * C2, p1[:], ALU.mult, ALU.add)
    x2p = ftile("x2p")
    nc.vector.tensor_mul(x2p[:], p1[:], rden[:])

    # x4p = (G*C1*C3*s + C1*c*f - AR*C3*x4 - C1^2*c*s*x4^2) / den
    p2 = ftile("p2")
    nc.vector.tensor_scalar(p2[:], s[:], G * C1 * C3, None, ALU.mult)
    nc.vector.scalar_tensor_tensor(p2[:], cf[:], C1, p2[:], ALU.mult, ALU.add)
    nc.vector.scalar_tensor_tensor(p2[:], x4, -AR * C3, p2[:], ALU.mult, ALU.add)
    nc.vector.scalar_tensor_tensor(p2[:], csx4sq[:], -C1SQ, p2[:], ALU.mult, ALU.add)
    x4p = ftile("x4p")
    nc.vector.tensor_mul(x4p[:], p2[:], rden[:])

    # Vdot = dV . [x2, x2p, x4, x4p]
    vd = ftile("vd")
    nc.vector.tensor_mul(vd[:], dv0[:], x2)
    t_ = ftile("vt")
    nc.vector.tensor_mul(t_[:], dv1[:], x2p[:])
    nc.vector.tensor_add(vd[:], vd[:], t_[:])
    nc.vector.tensor_mul(t_[:], dv2[:], x4)
    nc.vector.tensor_add(vd[:], vd[:], t_[:])
    nc.vector.tensor_mul(t_[:], dv3[:], x4p[:])
    nc.vector.tensor_add(vd[:], vd[:], t_[:])

    # penalties: PEN*relu(-V) + PEN*relu(Vdot)
    pen = ftile("pen")
    nc.vector.tensor_scalar(pen[:], vpl[:], 0.0, -PEN, ALU.min, ALU.mult)
    pen2 = ftile("pen2")
    nc.vector.tensor_scalar(pen2[:], vd[:], 0.0, PEN, ALU.max, ALU.mult)
    nc.vector.tensor_add(pen[:], pen[:], pen2[:])
    nc.sync.dma_start(
        loss_out[s0:s0 + BCH].rearrange("(p f) -> p f", p=128), pen[:])

    # partial sums for custom_loss: sum(d^2), sum((y - y_pred)^2)
    ypc = ftile("ypc")
    nc.vector.tensor_scalar(ypc[:], ypred[:], EPS, None, ALU.max)
    l1 = ftile("l1")
    nc.scalar.activation(l1[:], ypc[:], AF.Ln, bias=1.0)
    yc = ftile("yc")
    nc.vector.tensor_scalar(yc[:], ypl[:], EPS, None, ALU.max)
    l2 = ftile("l2")
    nc.scalar.activation(l2[:], yc[:], AF.Ln, bias=1.0)
    dd = ftile("dd")
    nc.vector.tensor_sub(dd[:], l1[:], l2[:])
    d2s = fpool.tile([128, 1], F32, tag="d2s", name="d2s")
    dtmp = ftile("dtmp")
    nc.scalar.activation(dtmp[:], dd[:], AF.Square, bias=zc[:], accum_out=d2s[:])
    ee = ftile("ee")
    nc.vector.tensor_sub(ee[:], ypl[:], ypred[:])
    es = fpool.tile([128, 1], F32, tag="es", name="es")
    nc.scalar.activation(dtmp[:], ee[:], AF.Square, bias=zc[:], accum_out=es[:])

    parts = fpool.tile([128, 2], F32, tag="parts" + str(half),
                       name="parts" + str(half))
    nc.vector.tensor_copy(parts[:, 0:1], d2s[:])
    nc.vector.tensor_copy(parts[:, 1:2], es[:])
    nc.sync.dma_start(part_out[:, 2 * half:2 * half + 2], parts[:])


def kernel(**inputs):
    x = np.ascontiguousarray(inputs["x"], dtype=np.float32)
    y = np.ascontiguousarray(inputs["y"], dtype=np.float32)
    W1 = np.ascontiguousarray(inputs["W1"], dtype=np.float32)
    b1 = np.ascontiguousarray(inputs["b1"], dtype=np.float32)
    W2 = np.ascontiguousarray(inputs["W2"], dtype=np.float32)
    b2 = np.ascontiguousarray(inputs["b2"], dtype=np.float32)
    W3 = np.ascontiguousarray(inputs["W3"], dtype=np.float32)
    b3 = np.ascontiguousarray(inputs["b3"], dtype=np.float32)

    if "nc" not in _NC_CACHE:
        _NC_CACHE["nc"] = build()
    nc = _NC_CACHE["nc"]

    in_maps = []
    for cid in range(NCORES):
        sl = slice(cid * BC, (cid + 1) * BC)
        in_maps.append({
            "x": x[sl], "y": y[sl],
            "W1": W1, "b1": b1, "W2": W2, "b2": b2, "W3": W3, "b3": b3,
        })
    res = run_bass_kernel_spmd(nc, in_maps, list(range(NCORES)))

    loss = np.concatenate([res.results[c]["loss_pen"] for c in range(NCORES)])
    parts = np.stack([res.results[c]["partials"] for c in range(NCORES)])
    sums = parts.astype(np.float64).sum(axis=(0, 1))
    scalar = (ALPHA * (sums[0] + sums[2]) / B
              + (1.0 - ALPHA) * (sums[1] + sums[3]) / B)
    return (loss + np.float32(scalar)).astype(np.float32)
